# revision 15
# baseline (speedup 1.0000x reference)
"""DBPNet Trainium2 kernel: 8-core data-parallel Bass/Tile implementation.

v2 scheme:
  - ADMM algebra: with M = (AAH + rho I)^-1,  AAH@M = I - rho*M, so
      tmv = marc + rho*c1*zmu - rho^2*c1*mz      (mz = M@zmu, marc = M@arc)
      Ax  = rho*tmv   (no AAH matmuls on device at all)
      x   = rc1 + A^H(rho^2*c1*mz - marc)
    -> one 8-matmul group (mz) per ADMM step.
  - proj state: u' = (1-fac)*diff; zmu'(within iter) = y + (2fac-1)*diff;
    zmu0(next iter) = y + (fac-1)*diff.
  - combo transposes via wide signed identities [I|J]: one PE op -> top+bottom.
  - CNN in bf16 single-tile activations [128, 4bp, 2050], block-diag weights.
  - arc path (AT stream + rcT) in bf16; everything else f32r.
  - BN stats pre-reduced to [32,2] and packed [2,32] before the collective
    (2/16-descriptor DMAs); AllGather + local reduce.
"""
import numpy as np

B, Nv, Nt, F = 128, 512, 2048, 32
NCORE, BS = 8, 16
ITERS, ADMM = 5, 3
BN_EPS = 1e-5
USE_AG = True


# ---------------------------------------------------------------- host prep
def _host_prep(inputs):
    import ml_dtypes
    A = np.ascontiguousarray(np.asarray(inputs['A'], np.float32))
    Ar, Ai = A[0], A[1]
    Ac = Ar.astype(np.float64) + 1j * Ai.astype(np.float64)
    AAH = Ac @ Ac.conj().T

    rhos = np.exp(np.asarray(inputs['log_rho'], np.float32)).astype(np.float32)
    epss = np.exp(np.asarray(inputs['log_eps'], np.float32)).astype(np.float32)

    minv_stacks, rho_to_idx, iter_minv_idx = [], {}, []
    for r in rhos:
        key = float(r)
        if key not in rho_to_idx:
            M = np.linalg.inv(AAH + key * np.eye(Nv))
            Mr = M.real.astype(np.float32)
            Mi = M.imag.astype(np.float32)
            mstk = np.concatenate([Mr.T, Mi.T], 0).reshape(8, 128, 512)
            mstk = mstk[[0, 4, 1, 5, 2, 6, 3, 7]]
            minv_stacks.append(
                mstk.transpose(1, 0, 2).copy().astype(ml_dtypes.bfloat16))
            rho_to_idx[key] = len(minv_stacks) - 1
        iter_minv_idx.append(rho_to_idx[float(r)])
    nu = len(minv_stacks)

    A1 = np.concatenate([Ar, Ai], 0)                    # [1024, 2048]
    AB = A1.reshape(8, 128, 2048)[[0, 4, 1, 5, 2, 6, 3, 7]]
    AB = AB.transpose(1, 0, 2).copy().astype(ml_dtypes.bfloat16)
    AT1 = np.concatenate([Ar.T, Ai.T], 0)               # [4096, 512]
    ATR = AT1.reshape(32, 128, 512)
    ilv = [x for tc in range(16) for x in (tc, 16 + tc)]
    ATR = ATR[ilv].transpose(1, 0, 2).copy().astype(ml_dtypes.bfloat16)

    # bf16 CNN weights pack [128, 896]
    w1 = np.asarray(inputs['conv1_w'], np.float32)
    w2 = np.asarray(inputs['conv2_w'], np.float32)
    wf = np.asarray(inputs['convf_w'], np.float32)
    W1 = np.zeros((128, 128), np.float32)
    for dl in range(3):
        for ci in range(2):
            for q in range(4):
                W1[dl * 8 + ci * 4 + q, np.arange(F) * 4 + q] = w1[:, ci, dl]
    W2 = np.zeros((3, 128, 128), np.float32)
    for dl in range(3):
        for ci in range(F):
            for q in range(4):
                W2[dl, ci * 4 + q, np.arange(F) * 4 + q] = w2[:, ci, dl]
    WFb = np.zeros((4, 3, 128, 32), np.float32)
    for bp in range(4):
        for dl in range(3):
            for ci in range(F):
                for q in range(4):
                    for cf in range(2):
                        WFb[bp, dl, ci * 4 + q, cf * 16 + bp * 4 + q] = wf[cf, ci, dl]
    WTB = np.concatenate(
        [W1] + [W2[d] for d in range(3)]
        + [WFb[bp, dl] for bp in range(4) for dl in range(3)],
        axis=1).astype(ml_dtypes.bfloat16)              # [128, 896]

    # f32 selector/identity pack
    OSEL = np.zeros((128, 32), np.float32)
    SELB = np.zeros((128, 128), np.float32)   # rows 0-31 used
    for co in range(32):
        for q in range(4):
            OSEL[co * 4 + q, co] = 1.0
            SELB[co, co * 4 + q] = 1.0
    I16 = np.eye(16, dtype=np.float32)
    JN = np.zeros((32, 32), np.float32)
    JN[16:, :16] = -I16
    JN[:16, 16:] = I16
    JH = -JN
    ID32 = np.eye(32, dtype=np.float32)
    IDJN = np.zeros((128, 64), np.float32)
    IDJN[:32, :32] = ID32
    IDJN[:32, 32:] = JN
    IDJH = np.zeros((128, 64), np.float32)
    IDJH[:32, :32] = ID32
    IDJH[:32, 32:] = JH
    P32 = np.zeros((128, 32), np.float32)
    for m in range(32):
        P32[m, m] = 1.0
        P32[(m + 16) % 32, m] = 1.0
    ID32p = np.zeros((128, 32), np.float32)
    ID32p[:32, :32] = ID32
    ONESR = np.zeros((128, 128), np.float32)
    ONESR[0, :] = 1.0
    WTS = np.concatenate([OSEL, SELB, IDJN, IDJH, P32, ID32p, ONESR], axis=1)

    g1 = np.asarray(inputs['bn1_g'], np.float32)
    b1 = np.asarray(inputs['bn1_b'], np.float32)
    g2 = np.asarray(inputs['bn2_g'], np.float32)
    b2 = np.asarray(inputs['bn2_b'], np.float32)
    fb = np.asarray(inputs['convf_b'], np.float32)
    CF = np.zeros((128, 8), np.float32)
    CF[:32, 0] = g1
    CF[:32, 1] = b1
    CF[:32, 2] = g2
    CF[:32, 3] = b2
    CF[:, 6] = BN_EPS
    FBC = np.zeros((128, nu), np.float32)
    for key, idx in rho_to_idx.items():
        c1 = 1.0 / (key + 1e-8)
        FBC[:16, idx] = fb[0] * c1
        FBC[16:32, idx] = fb[1] * c1

    y = np.asarray(inputs['y'], np.float32)
    ybm_cores, ylh_cores, yln_cores = [], [], []
    for c in range(NCORE):
        ys = y[c * BS:(c + 1) * BS]
        ybm = np.concatenate([ys[:, 0], ys[:, 1]], 0)    # [32, Nv]
        ybm_cores.append(np.ascontiguousarray(ybm))
        sT = ybm.T                                       # [Nv, 32]
        botH = np.concatenate([sT[:, 16:], -sT[:, :16]], 1)
        botN = np.concatenate([-sT[:, 16:], sT[:, :16]], 1)
        comboH = np.concatenate([sT, botH], 0)           # [2Nv, 32]
        comboN = np.concatenate([sT, botN], 0)
        ylh_cores.append(comboH.reshape(8, 128, 32)[[0, 4, 1, 5, 2, 6, 3, 7]]
                         .transpose(1, 0, 2).copy().astype(ml_dtypes.bfloat16))
        yln_cores.append(comboN.reshape(8, 128, 32)[[0, 4, 1, 5, 2, 6, 3, 7]]
                         .transpose(1, 0, 2).copy().astype(ml_dtypes.bfloat16))

    return dict(AB=AB, ATR=ATR, minv_stacks=minv_stacks,
                iter_minv_idx=iter_minv_idx, rhos=rhos, epss=epss,
                WTS=WTS, WTB=WTB, CF=CF, FBC=FBC, ybm_cores=ybm_cores,
                ylh_cores=ylh_cores, yln_cores=yln_cores)


# WTS column offsets
OSEL_C = 0
SELB_C = 32
IDJN_C = SELB_C + 128
IDJH_C = IDJN_C + 64
P32_C = IDJH_C + 64
ID32_C = P32_C + 32
ONES_C = ID32_C + 32
WTS_W = ONES_C + 128
# WTB column offsets
W1_C = 0
W2_C = 128
WF_C = W2_C + 384
WTB_W = WF_C + 384


# ---------------------------------------------------------------- program
def _build_program(prep):
    import concourse.bacc as bacc
    import concourse.tile as tile
    import concourse.mybir as mybir

    dt = mybir.dt
    f32, f32r, bf16 = dt.float32, dt.float32r, dt.bfloat16
    AX = mybir.AxisListType
    OP = mybir.AluOpType
    AF = mybir.ActivationFunctionType

    nu = len(prep['minv_stacks'])
    rhos, epss = prep['rhos'], prep['epss']
    cnt = float(B * Nt)

    nc = bacc.Bacc("TRN2", target_bir_lowering=False, debug=False,
                   num_devices=NCORE)

    AB_d = nc.dram_tensor("AB", [128, 8, 2048], bf16, kind="ExternalInput")
    ATR_d = nc.dram_tensor("ATR", [128, 32, 512], bf16, kind="ExternalInput")
    MINV_d = nc.dram_tensor("MINVS", [nu, 128, 8, 512], bf16, kind="ExternalInput")
    WTS_d = nc.dram_tensor("WTS", [128, WTS_W], f32r, kind="ExternalInput")
    WTB_d = nc.dram_tensor("WTB", [128, WTB_W], bf16, kind="ExternalInput")
    CF_d = nc.dram_tensor("CF", [128, 8], f32, kind="ExternalInput")
    FBC_d = nc.dram_tensor("FBC", [128, nu], f32, kind="ExternalInput")
    Y_d = nc.dram_tensor("YBM", [32, 512], f32r, kind="ExternalInput")
    YLH_d = nc.dram_tensor("YLH", [128, 8, 32], bf16, kind="ExternalInput")
    YLN_d = nc.dram_tensor("YLN", [128, 8, 32], bf16, kind="ExternalInput")
    XO_d = nc.dram_tensor("XOUT", [32, 2048], f32r, kind="ExternalOutput")

    with tile.TileContext(nc) as tc:
        with (
            tc.tile_pool(name="cst", bufs=1) as cst,
            tc.tile_pool(name="st", bufs=1) as stp,
            tc.tile_pool(name="cnn", bufs=1) as cnnp,
            tc.tile_pool(name="xin", bufs=4) as xinp,
            tc.tile_pool(name="psbig", bufs=2, space="PSUM") as psbig,
            tc.tile_pool(name="psgen", bufs=2, space="PSUM") as psgen,
            tc.tile_pool(name="psarc", bufs=1, space="PSUM") as psarc,
            tc.tile_pool(name="pssm", bufs=3, space="PSUM") as pssm,
            tc.tile_pool(name="ddr", bufs=2, space="DRAM") as ddr,
        ):
            # ---- constants ----
            ab = cst.tile([128, 8, 2048], bf16, tag="ab")
            atr = cst.tile([128, 32, 512], bf16, tag="atr")
            minv = cst.tile([128, 8, 512], bf16, tag="minv")
            wts = cst.tile([128, WTS_W], f32r, tag="wts")
            wtb = cst.tile([128, WTB_W], bf16, tag="wtb")
            cf = cst.tile([128, 8], f32, tag="cf")
            fbc = cst.tile([128, nu], f32, tag="fbc")
            ylh = cst.tile([128, 8, 32], bf16, tag="ylh")
            yln = cst.tile([128, 8, 32], bf16, tag="yln")
            nc.sync.dma_start(ab[:], AB_d[:])
            nc.sync.dma_start(atr[:], ATR_d[:])
            nc.sync.dma_start(wts[:], WTS_d[:])
            nc.sync.dma_start(wtb[:], WTB_d[:])
            nc.sync.dma_start(cf[:], CF_d[:])
            nc.sync.dma_start(fbc[:], FBC_d[:])
            nc.sync.dma_start(ylh[:], YLH_d[:])
            nc.sync.dma_start(yln[:], YLN_d[:])
            if nu == 1:
                nc.sync.dma_start(minv[:], MINV_d[0])

            OSEL = wts[:, OSEL_C:OSEL_C + 32]
            SELB = wts[0:32, SELB_C:SELB_C + 128]
            IDJN = wts[0:32, IDJN_C:IDJN_C + 64]
            IDJH = wts[0:32, IDJH_C:IDJH_C + 64]
            P32 = wts[0:32, P32_C:P32_C + 32]
            ID32 = wts[0:32, ID32_C:ID32_C + 32]
            ONE128 = wts[0:1, ONES_C:ONES_C + 128]
            ONE11 = wts[0:1, ONES_C:ONES_C + 1]
            W1 = wtb[0:24, W1_C:W1_C + 128]
            W2 = [wtb[:, W2_C + 128 * d: W2_C + 128 * (d + 1)] for d in range(3)]
            WF = [[wtb[:, WF_C + (bp * 3 + d) * 32: WF_C + (bp * 3 + d) * 32 + 32]
                   for d in range(3)] for bp in range(4)]
            g32 = [cf[0:32, 0:1], cf[0:32, 2:3]]
            b32 = [cf[0:32, 1:2], cf[0:32, 3:4]]
            zb128 = cf[:, 5:6]
            zb32 = cf[0:32, 5:6]
            zb1 = cf[0:1, 5:6]
            epsb = cf[0:32, 6:7]

            # ---- state ----
            X = stp.tile([32, 2048], f32r, tag="X")
            RC = stp.tile([32, 2048], f32r, tag="RC")
            XB = stp.tile([32, 2050], bf16, tag="XB")
            S = stp.tile([32, 8, 512], f32r, tag="S")
            y_t, qb_t, y2_t, marc_t = S[:, 0, :], S[:, 1, :], S[:, 2, :], S[:, 3, :]
            q3_t, t1_t, dif_t, sq_t = S[:, 4, :], S[:, 5, :], S[:, 6, :], S[:, 7, :]
            arcS = stp.tile([32, 512], f32r, tag="arcS")
            mzS = stp.tile([32, 512], f32r, tag="mzS")
            s32f = stp.tile([32, 1], f32, tag="s32f")
            facs = stp.tile([32, 4], f32, tag="facs")
            smal = stp.tile([32, 8], f32r, tag="smal")
            gbt = stp.tile([32, 2], f32r, tag="gbt")
            gbb = stp.tile([128, 2], f32, tag="gbb")
            zmuT = stp.tile([128, 8, 32], bf16, tag="zmuT")
            difT = stp.tile([128, 8, 32], bf16, tag="difT")
            wT = stp.tile([128, 8, 32], bf16, tag="wT")
            frow = stp.tile([1, 8, 32], f32, tag="frow")
            g0sb = stp.tile([32, 1], f32, tag="g0sb")
            rcT = stp.tile([128, 32, 32], bf16, tag="rcT")
            stat = cnnp.tile([128, 32], f32, tag="stat")
            stat2 = cnnp.tile([128, 2], f32, tag="stat2")
            stat2r = cnnp.tile([128, 2], f32r, tag="stat2r")
            p32s = cnnp.tile([32, 2], f32r, tag="p32s")
            sdma = cnnp.tile([2, 32], f32, tag="sdma")
            st2 = cnnp.tile([2, 32], f32r, tag="st2")
            agg = cnnp.tile([2, 8, 32], f32, tag="agg")

            A1 = cnnp.tile([128, 4, 2050], bf16, tag="A1")
            A2 = cnnp.tile([128, 4, 2050], bf16, tag="A2")
            SQD = cnnp.tile([128, 2, 512], bf16, tag="SQD")

            nc.sync.dma_start(y_t[:], Y_d[:])
            nc.vector.memset(XB[:, 0:1].bitcast(dt.uint16), 0)
            nc.vector.memset(XB[:, 2049:2050].bitcast(dt.uint16), 0)
            for a in (A1, A2):
                nc.vector.memset(a[:, :, 0:1].bitcast(dt.uint16), 0)
                nc.vector.memset(a[:, :, 2049:2050].bitcast(dt.uint16), 0)

            # ---- x0 = A^H y ----
            for ntb in range(4):
                p = psgen.tile([32, 512], f32, tag="mm")
                for k in range(8):
                    nc.tensor.matmul(p[:], ylh[:, k, :],
                                     ab[:, k, 512 * ntb:512 * (ntb + 1)],
                                     start=(k == 0), stop=(k == 7))
                nc.vector.tensor_copy(X[:, 512 * ntb:512 * (ntb + 1)], p[:])

            def combo_pair(dst, pair, src_view, idj):
                # plain matmul: out = src^T @ [I|J] (is_transpose ignores the
                # identity's values, so signed J requires the normal datapath);
                # top/bottom slots adjacent -> single cast copy
                pT = pssm.tile([128, 64], f32, tag="sm")
                nc.tensor.matmul(pT[:], src_view, idj, start=True, stop=True)
                nc.vector.tensor_copy(
                    dst[:, 2 * pair:2 * pair + 2, :],
                    pT[:].rearrange("p (a b) -> p a b", a=2))

            def bn_collective(layer, acts):
                with nc.allow_low_precision(reason="bn stat reduce"):
                    nc.vector.tensor_reduce(stat2[:, 0:1], stat[:, 0:16], AX.X, OP.add)
                    nc.vector.tensor_reduce(stat2[:, 1:2], stat[:, 16:32], AX.X, OP.add)
                nc.vector.tensor_copy(stat2r[:], stat2[:])
                p = pssm.tile([32, 2], f32, tag="sm")
                nc.tensor.matmul(p[:], OSEL, stat2r[:], start=True, stop=True)
                with nc.allow_low_precision(reason="pack"):
                    nc.vector.tensor_copy(p32s[:], p[:])
                pt = pssm.tile([2, 32], f32, tag="sm")
                nc.tensor.matmul(pt[:], p32s[:], ID32, start=True, stop=True)
                nc.vector.tensor_copy(sdma[:], pt[:])
                ci_ = ddr.tile([2, 32], f32, tag="cc")
                if USE_AG:
                    co_ = ddr.tile([8, 2, 32], f32, tag="cc2")
                    nc.sync.dma_start(ci_[:], sdma[:])
                    nc.gpsimd.collective_compute(
                        "AllGather", OP.bypass,
                        replica_groups=[list(range(NCORE))],
                        ins=[ci_.opt()], outs=[co_.opt()])
                    nc.sync.dma_start(agg[:], co_[:].rearrange("c s v -> s c v"))
                    with nc.allow_low_precision(reason="bn stat reduce"):
                        nc.vector.tensor_reduce(
                            st2[:], agg[:].rearrange("s c v -> s v c"),
                            AX.X, OP.add)
                else:
                    co1 = ddr.tile([2, 32], f32, tag="cc3")
                    nc.sync.dma_start(ci_[:], sdma[:])
                    nc.gpsimd.collective_compute(
                        "AllReduce", OP.add,
                        replica_groups=[list(range(NCORE))],
                        ins=[ci_.opt()], outs=[co1.opt()])
                    nc.sync.dma_start(st2[:].bitcast(f32), co1[:])
                pb = pssm.tile([32, 32], f32, tag="sm")
                nc.tensor.matmul(pb[:, 0:2], st2[:], ID32[0:2, 0:2],
                                 start=True, stop=True)
                with nc.allow_low_precision(reason="bn scalar math in f32r"):
                    nc.vector.tensor_scalar_mul(smal[:, 0:1], pb[:, 0:1], 1.0 / cnt)
                    nc.vector.tensor_scalar_mul(smal[:, 1:2], pb[:, 1:2], 1.0 / cnt)
                    nc.vector.tensor_mul(smal[:, 2:3], smal[:, 0:1], smal[:, 0:1])
                    nc.vector.tensor_sub(smal[:, 3:4], smal[:, 1:2], smal[:, 2:3])
                    nc.scalar.activation(smal[:, 3:4], smal[:, 3:4], AF.Sqrt,
                                         bias=epsb)
                    nc.vector.reciprocal(smal[:, 3:4], smal[:, 3:4])
                    nc.vector.tensor_mul(gbt[:, 0:1], g32[layer], smal[:, 3:4])
                    nc.vector.tensor_mul(smal[:, 2:3], smal[:, 0:1], gbt[:, 0:1])
                    nc.vector.tensor_sub(gbt[:, 1:2], b32[layer], smal[:, 2:3])
                p2 = pssm.tile([128, 2], f32, tag="sm")
                nc.tensor.matmul(p2[:], SELB, gbt[:], start=True, stop=True)
                nc.vector.tensor_copy(gbb[:], p2[:])
                for bp in range(2):
                    nc.scalar.activation(acts[:, bp, 1:2049], acts[:, bp, 1:2049],
                                         AF.Relu, bias=gbb[:, 1:2],
                                         scale=gbb[:, 0:1])
                for bp in range(2, 4):
                    av = acts[:, bp, 1:2049]
                    nc.vector.tensor_scalar(av, av, gbb[:, 0:1], gbb[:, 1:2],
                                            OP.mult, OP.add)
                    nc.vector.tensor_scalar(av, av, 0.0, None, OP.max)

            # ================= iterations =================
            for it in range(ITERS):
                rho = float(rhos[it])
                eps = float(epss[it])
                c1 = 1.0 / (rho + 1e-8)
                uidx = prep['iter_minv_idx'][it]
                if nu > 1:
                    nc.sync.dma_start(minv[:], MINV_d[uidx])

                # ---- CNN: xb cast + gather ----
                nc.vector.tensor_copy(XB[:, 1:2049], X[:])
                xins = []
                for bp in range(4):
                    xin = xinp.tile([24, 2048], bf16, tag="xin")
                    for dl in range(3):
                        for ci in range(2):
                            eng = nc.sync if (dl % 2 == 0) else nc.scalar
                            eng.dma_start(
                                xin[dl * 8 + ci * 4: dl * 8 + ci * 4 + 4, :],
                                XB[ci * 16 + bp * 4: ci * 16 + bp * 4 + 4,
                                   dl:dl + 2048])
                    xins.append(xin)
                # conv1
                for lt in range(4):
                    for bp in range(4):
                        pb_ = psbig.tile([128, 512], f32, tag="big")
                        nc.tensor.matmul(
                            pb_[:], W1,
                            xins[bp][0:24, 512 * lt:512 * (lt + 1)],
                            start=True, stop=True)
                        g = lt * 4 + bp
                        a1v = A1[:, bp, 1 + 512 * lt:1 + 512 * (lt + 1)]
                        nc.scalar.activation(a1v, pb_[:], AF.Identity,
                                             bias=zb128,
                                             accum_out=stat[:, g:g + 1])
                        nc.vector.scalar_tensor_tensor(
                            SQD[:, 0, :], a1v, 1.0, a1v, OP.mult, OP.mult,
                            accum_out=stat[:, 16 + g:17 + g])

                # step-0 mz: runs on PE during the bn1 collective stall
                lhs0 = yln if it == 0 else zmuT
                pmz0 = psgen.tile([32, 512], f32, tag="mm")
                for k in range(8):
                    nc.tensor.matmul(pmz0[:], lhs0[:, k, :], minv[:, k, :],
                                     start=(k == 0), stop=(k == 7))
                nc.vector.tensor_copy(mzS[:], pmz0[:])

                bn_collective(0, A1)

                # conv2
                for lt in range(4):
                    for bp in range(4):
                        pb_ = psbig.tile([128, 512], f32, tag="big")
                        for dl in range(3):
                            nc.tensor.matmul(
                                pb_[:], W2[dl],
                                A1[:, bp, dl + 512 * lt: dl + 512 * lt + 512],
                                start=(dl == 0), stop=(dl == 2))
                        g = lt * 4 + bp
                        a2v = A2[:, bp, 1 + 512 * lt:1 + 512 * (lt + 1)]
                        nc.scalar.activation(a2v, pb_[:], AF.Identity,
                                             bias=zb128,
                                             accum_out=stat[:, g:g + 1])
                        nc.vector.scalar_tensor_tensor(
                            SQD[:, 0, :], a2v, 1.0, a2v, OP.mult, OP.mult,
                            accum_out=stat[:, 16 + g:17 + g])
                bn_collective(1, A2)

                # convf + rc1 + comboN(rc1) + arc matmuls
                parc = psarc.tile([32, 512], f32, tag="arc")
                first_mm = True
                for lt in range(4):
                    pcf = psgen.tile([32, 512], f32, tag="mm")
                    nmm = 0
                    for bp in range(4):
                        for dl in range(3):
                            nc.tensor.matmul(
                                pcf[:], WF[bp][dl],
                                A2[:, bp, dl + 512 * lt: dl + 512 * lt + 512],
                                start=(nmm == 0), stop=(nmm == 11))
                            nmm += 1
                    blk = slice(512 * lt, 512 * (lt + 1))
                    # rc1 = (pcf + x)*c1 + fb*c1
                    nc.vector.scalar_tensor_tensor(
                        sq_t[:], pcf[:], 1.0, X[:, blk], OP.mult, OP.add)
                    nc.scalar.activation(RC[:, blk], sq_t[:], AF.Identity,
                                         bias=fbc[0:32, uidx:uidx + 1], scale=c1)
                    for c in range(4):
                        combo_pair(rcT, lt * 4 + c,
                                   RC[:, 512 * lt + 128 * c:
                                      512 * lt + 128 * (c + 1)], IDJN)
                    for c in range(4):
                        tc_ = lt * 4 + c
                        nc.tensor.matmul(parc[:], rcT[:, 2 * tc_, :],
                                         atr[:, 2 * tc_, :],
                                         start=first_mm, stop=False)
                        first_mm = False
                        last = (lt == 3 and c == 3)
                        nc.tensor.matmul(parc[:], rcT[:, 2 * tc_ + 1, :],
                                         atr[:, 2 * tc_ + 1, :],
                                         start=False, stop=last)
                nc.vector.tensor_copy(arcS[:], parc[:])
                # comboN(arc) -> wT scratch; marc = Minv @ arc
                for c in range(4):
                    combo_pair(wT, c, arcS[:, 128 * c:128 * (c + 1)], IDJN)
                pma = psgen.tile([32, 512], f32, tag="mm")
                for k in range(8):
                    nc.tensor.matmul(pma[:], wT[:, k, :], minv[:, k, :],
                                     start=(k == 0), stop=(k == 7))
                nc.vector.tensor_copy(marc_t[:], pma[:])

                # ---------- ADMM steps ----------
                r2c1 = rho * rho * c1
                r3c1 = rho * rho * rho * c1
                # qbase = rho*marc + (rho^2*c1 - 1)*y ; y2 precomputed
                nc.vector.tensor_scalar(y2_t[:], y_t[:], r2c1 - 1.0, None,
                                        OP.mult)
                nc.vector.scalar_tensor_tensor(
                    qb_t[:], marc_t[:], rho, y2_t[:], OP.mult, OP.add)
                if it > 0:
                    # q3(s0) = qbase + gamma0*diff_last
                    nc.vector.scalar_tensor_tensor(
                        q3_t[:], dif_t[:], g0sb[:], qb_t[:], OP.mult, OP.add)
                for s in range(ADMM):
                    final = (s == ADMM - 1)
                    last_all = (it == ITERS - 1 and final)
                    # mz
                    if s == 0:
                        mz_ap = mzS[:]
                    else:
                        pmz = psgen.tile([32, 512], f32, tag="mm")
                        for k in range(8):
                            nc.tensor.matmul(pmz[:], zmuT[:, k, :], minv[:, k, :],
                                             start=(k == 0), stop=(k == 7))
                        mz_ap = pmz[:]
                    # diff = q3 - rho^3*c1*mz
                    qsrc = qb_t if (it == 0 and s == 0) else q3_t
                    nc.vector.scalar_tensor_tensor(
                        dif_t[:], mz_ap, -r3c1, qsrc[:], OP.mult, OP.add)
                    if final:
                        nc.vector.scalar_tensor_tensor(
                            t1_t[:], mz_ap, r2c1, marc_t[:], OP.mult, OP.subtract)
                    # diffT combo (PE, overlaps norm chain)
                    if not last_all:
                        for c in range(4):
                            combo_pair(difT, c,
                                       dif_t[:, 128 * c:128 * (c + 1)], IDJN)
                    # norm chain in [1,32] row space
                    nc.vector.scalar_tensor_tensor(
                        sq_t[:], dif_t[:], 1.0, dif_t[:], OP.mult, OP.mult,
                        accum_out=s32f[:])
                    pn = pssm.tile([1, 32], f32, tag="sm")
                    nc.tensor.matmul(pn[:], s32f[:], P32.bitcast(f32),
                                     start=True, stop=True)
                    nc.scalar.activation(frow[0:1, 0, :], pn[:], AF.Sqrt,
                                         bias=zb1)
                    nc.vector.reciprocal(frow[0:1, 0, :], frow[0:1, 0, :])
                    nc.vector.tensor_scalar(frow[0:1, 1, :], frow[0:1, 0, :],
                                            eps, 1.0, OP.mult, OP.min)
                    if final:
                        ga, gb = r2c1 - 1.0, 1.0 - r2c1   # gamma0 (z reset)
                        za, zbs = 1.0, -1.0               # zfac = fac - 1
                    else:
                        ga, gb = 2.0 * r2c1 - 1.0, 1.0 - r2c1
                        za, zbs = 2.0, -1.0               # zfac = 2fac - 1
                    nc.vector.tensor_scalar(frow[0:1, 2, :], frow[0:1, 1, :],
                                            za, zbs, OP.mult, OP.add)
                    if not last_all:
                        nc.vector.tensor_scalar(frow[0:1, 3, :], frow[0:1, 1, :],
                                                ga, gb, OP.mult, OP.add)
                        # broadcast zfac across partitions
                        pfb = pssm.tile([128, 1, 32], f32, tag="sm")
                        nc.tensor.matmul(pfb[:, 0, :], ONE128.bitcast(f32),
                                         frow[0:1, 2, :], start=True, stop=True)
                        # zmuT' = yln + zfac (x) diffT
                        fbv = pfb[:].broadcast_to([128, 8, 32])
                        nc.vector.tensor_mul(zmuT[:], difT[:], fbv)
                        nc.vector.tensor_add(zmuT[:], zmuT[:], yln[:])
                        # gamma to B-space column
                        pg = pssm.tile([32, 1], f32, tag="sm")
                        nc.tensor.matmul(pg[:], frow[0:1, 3, :],
                                         ONE11.bitcast(f32), start=True,
                                         stop=True)
                        if final:
                            nc.vector.tensor_copy(g0sb[:], pg[:])
                        else:
                            nc.vector.scalar_tensor_tensor(
                                q3_t[:], dif_t[:], pg[:], qb_t[:],
                                OP.mult, OP.add)
                    if final:
                        for c in range(4):
                            combo_pair(wT, c,
                                       t1_t[:, 128 * c:128 * (c + 1)], IDJH)
                        for ntb in range(4):
                            px = psgen.tile([32, 512], f32, tag="mm")
                            for k in range(8):
                                nc.tensor.matmul(
                                    px[:], wT[:, k, :],
                                    ab[:, k, 512 * ntb:512 * (ntb + 1)],
                                    start=(k == 0), stop=(k == 7))
                            blk = slice(512 * ntb, 512 * (ntb + 1))
                            nc.vector.tensor_add(X[:, blk], px[:], RC[:, blk])

            nc.sync.dma_start(XO_d[:], X[:])

    nc.compile()
    return nc


def _enable_trace_shim():
    import sys, types
    try:
        import trn_agent_boot.trn_boot as _tb
        import concourse.bass_utils as _bu
        _bu.upload_artifacts = lambda tmpdir: "local://" + str(tmpdir)
        hookmod = types.ModuleType('antenv.axon_hooks')
        hook = _tb._ntff_profile_via_ctypes('/opt/axon/libaxon_pjrt.so')
        hookmod.get_axon_ntff_profile_hook = lambda: hook
        import antenv as _antenv
        sys.modules['antenv.axon_hooks'] = hookmod
        _antenv.axon_hooks = hookmod
        return True
    except Exception:
        return False


def kernel(**inputs) -> np.ndarray:
    import os
    from concourse.bass_utils import run_bass_kernel_spmd
    trace = bool(os.environ.get("KERNEL_TRACE"))
    if trace:
        trace = _enable_trace_shim()

    prep = _host_prep(inputs)
    nc = _build_program(prep)

    minvs = np.stack(prep['minv_stacks'], 0)
    in_maps = []
    for c in range(NCORE):
        in_maps.append({
            "AB": prep['AB'], "ATR": prep['ATR'], "MINVS": minvs,
            "WTS": prep['WTS'], "WTB": prep['WTB'], "CF": prep['CF'],
            "FBC": prep['FBC'],
            "YBM": np.ascontiguousarray(prep['ybm_cores'][c][:, :512]),
            "YLH": prep['ylh_cores'][c], "YLN": prep['yln_cores'][c],
        })
    res = run_bass_kernel_spmd(nc, in_maps, list(range(NCORE)), trace=trace)
    out = np.zeros((B, 2, Nt), np.float32)
    for c in range(NCORE):
        xc = res.results[c]["XOUT"]
        out[c * BS:(c + 1) * BS, 0] = xc[:16]
        out[c * BS:(c + 1) * BS, 1] = xc[16:]
    kernel._last_results = res
    return out


# revision 17
# speedup vs baseline: 1.0889x; 1.0889x over previous
"""DBPNet Trainium2 kernel: 8-core data-parallel Bass/Tile implementation.

v2 scheme:
  - ADMM algebra: with M = (AAH + rho I)^-1,  AAH@M = I - rho*M, so
      tmv = marc + rho*c1*zmu - rho^2*c1*mz      (mz = M@zmu, marc = M@arc)
      Ax  = rho*tmv   (no AAH matmuls on device at all)
      x   = rc1 + A^H(rho^2*c1*mz - marc)
    -> one 8-matmul group (mz) per ADMM step.
  - proj state: u' = (1-fac)*diff; zmu'(within iter) = y + (2fac-1)*diff;
    zmu0(next iter) = y + (fac-1)*diff.
  - combo transposes via wide signed identities [I|J]: one PE op -> top+bottom.
  - CNN in bf16 single-tile activations [128, 4bp, 2050], block-diag weights.
  - arc path (AT stream + rcT) in bf16; everything else f32r.
  - BN stats pre-reduced to [32,2] and packed [2,32] before the collective
    (2/16-descriptor DMAs); AllGather + local reduce.
"""
import numpy as np

B, Nv, Nt, F = 128, 512, 2048, 32
NCORE, BS = 8, 16
ITERS, ADMM = 5, 3
BN_EPS = 1e-5
USE_AG = True


# ---------------------------------------------------------------- host prep
def _host_prep(inputs):
    import ml_dtypes
    A = np.ascontiguousarray(np.asarray(inputs['A'], np.float32))
    Ar, Ai = A[0], A[1]
    Ac = Ar.astype(np.float64) + 1j * Ai.astype(np.float64)
    AAH = Ac @ Ac.conj().T

    rhos = np.exp(np.asarray(inputs['log_rho'], np.float32)).astype(np.float32)
    epss = np.exp(np.asarray(inputs['log_eps'], np.float32)).astype(np.float32)

    minv_stacks, rho_to_idx, iter_minv_idx = [], {}, []
    for r in rhos:
        key = float(r)
        if key not in rho_to_idx:
            M = np.linalg.inv(AAH + key * np.eye(Nv))
            Mr = M.real.astype(np.float32)
            Mi = M.imag.astype(np.float32)
            mstk = np.concatenate([Mr.T, Mi.T], 0).reshape(8, 128, 512)
            mstk = mstk[[0, 4, 1, 5, 2, 6, 3, 7]]
            minv_stacks.append(
                mstk.transpose(1, 0, 2).copy().astype(ml_dtypes.bfloat16))
            rho_to_idx[key] = len(minv_stacks) - 1
        iter_minv_idx.append(rho_to_idx[float(r)])
    nu = len(minv_stacks)

    A1 = np.concatenate([Ar, Ai], 0)                    # [1024, 2048]
    AB = A1.reshape(8, 128, 2048)[[0, 4, 1, 5, 2, 6, 3, 7]]
    AB = AB.transpose(1, 0, 2).copy().astype(ml_dtypes.bfloat16)
    AT1 = np.concatenate([Ar.T, Ai.T], 0)               # [4096, 512]
    ATR = AT1.reshape(32, 128, 512)
    ilv = [x for tc in range(16) for x in (tc, 16 + tc)]
    ATR = ATR[ilv].transpose(1, 0, 2).copy().astype(ml_dtypes.bfloat16)

    # bf16 CNN weights pack [128, 896]
    w1 = np.asarray(inputs['conv1_w'], np.float32)
    w2 = np.asarray(inputs['conv2_w'], np.float32)
    wf = np.asarray(inputs['convf_w'], np.float32)
    W1 = np.zeros((128, 128), np.float32)
    for dl in range(3):
        for ci in range(2):
            for q in range(4):
                W1[dl * 8 + ci * 4 + q, np.arange(F) * 4 + q] = w1[:, ci, dl]
    W2 = np.zeros((3, 128, 128), np.float32)
    for dl in range(3):
        for ci in range(F):
            for q in range(4):
                W2[dl, ci * 4 + q, np.arange(F) * 4 + q] = w2[:, ci, dl]
    WFb = np.zeros((4, 3, 128, 32), np.float32)
    for bp in range(4):
        for dl in range(3):
            for ci in range(F):
                for q in range(4):
                    for cf in range(2):
                        WFb[bp, dl, ci * 4 + q, cf * 16 + bp * 4 + q] = wf[cf, ci, dl]
    WTB = np.concatenate(
        [W1] + [W2[d] for d in range(3)]
        + [WFb[bp, dl] for bp in range(4) for dl in range(3)],
        axis=1).astype(ml_dtypes.bfloat16)              # [128, 896]

    # f32 selector/identity pack
    OSEL = np.zeros((128, 32), np.float32)
    SELB = np.zeros((128, 128), np.float32)   # rows 0-31 used
    for co in range(32):
        for q in range(4):
            OSEL[co * 4 + q, co] = 1.0
            SELB[co, co * 4 + q] = 1.0
    I16 = np.eye(16, dtype=np.float32)
    JN = np.zeros((32, 32), np.float32)
    JN[16:, :16] = -I16
    JN[:16, 16:] = I16
    JH = -JN
    ID32 = np.eye(32, dtype=np.float32)
    IDJN = np.zeros((128, 64), np.float32)
    IDJN[:32, :32] = ID32
    IDJN[:32, 32:] = JN
    IDJH = np.zeros((128, 64), np.float32)
    IDJH[:32, :32] = ID32
    IDJH[:32, 32:] = JH
    P32 = np.zeros((128, 32), np.float32)
    for m in range(32):
        P32[m, m] = 1.0
        P32[(m + 16) % 32, m] = 1.0
    ID32p = np.zeros((128, 32), np.float32)
    ID32p[:32, :32] = ID32
    ONESR = np.zeros((128, 128), np.float32)
    ONESR[0, :] = 1.0
    WTS = np.concatenate([OSEL, SELB, IDJN, IDJH, P32, ID32p, ONESR], axis=1)

    g1 = np.asarray(inputs['bn1_g'], np.float32)
    b1 = np.asarray(inputs['bn1_b'], np.float32)
    g2 = np.asarray(inputs['bn2_g'], np.float32)
    b2 = np.asarray(inputs['bn2_b'], np.float32)
    fb = np.asarray(inputs['convf_b'], np.float32)
    CF = np.zeros((128, 8), np.float32)
    CF[:32, 0] = g1
    CF[:32, 1] = b1
    CF[:32, 2] = g2
    CF[:32, 3] = b2
    CF[:, 6] = BN_EPS
    FBC = np.zeros((128, nu), np.float32)
    for key, idx in rho_to_idx.items():
        c1 = 1.0 / (key + 1e-8)
        FBC[:16, idx] = fb[0] * c1
        FBC[16:32, idx] = fb[1] * c1

    y = np.asarray(inputs['y'], np.float32)
    ybm_cores, ylh_cores, yln_cores = [], [], []
    for c in range(NCORE):
        ys = y[c * BS:(c + 1) * BS]
        ybm = np.concatenate([ys[:, 0], ys[:, 1]], 0)    # [32, Nv]
        ybm_cores.append(np.ascontiguousarray(ybm))
        sT = ybm.T                                       # [Nv, 32]
        botH = np.concatenate([sT[:, 16:], -sT[:, :16]], 1)
        botN = np.concatenate([-sT[:, 16:], sT[:, :16]], 1)
        comboH = np.concatenate([sT, botH], 0)           # [2Nv, 32]
        comboN = np.concatenate([sT, botN], 0)
        ylh_cores.append(comboH.reshape(8, 128, 32)[[0, 4, 1, 5, 2, 6, 3, 7]]
                         .transpose(1, 0, 2).copy().astype(ml_dtypes.bfloat16))
        yln_cores.append(comboN.reshape(8, 128, 32)[[0, 4, 1, 5, 2, 6, 3, 7]]
                         .transpose(1, 0, 2).copy().astype(ml_dtypes.bfloat16))

    return dict(AB=AB, ATR=ATR, minv_stacks=minv_stacks,
                iter_minv_idx=iter_minv_idx, rhos=rhos, epss=epss,
                WTS=WTS, WTB=WTB, CF=CF, FBC=FBC, ybm_cores=ybm_cores,
                ylh_cores=ylh_cores, yln_cores=yln_cores)


# WTS column offsets
OSEL_C = 0
SELB_C = 32
IDJN_C = SELB_C + 128
IDJH_C = IDJN_C + 64
P32_C = IDJH_C + 64
ID32_C = P32_C + 32
ONES_C = ID32_C + 32
WTS_W = ONES_C + 128
# WTB column offsets
W1_C = 0
W2_C = 128
WF_C = W2_C + 384
WTB_W = WF_C + 384


# ---------------------------------------------------------------- program
def _build_program(prep):
    import concourse.bacc as bacc
    import concourse.tile as tile
    import concourse.mybir as mybir

    dt = mybir.dt
    f32, f32r, bf16 = dt.float32, dt.float32r, dt.bfloat16
    AX = mybir.AxisListType
    OP = mybir.AluOpType
    AF = mybir.ActivationFunctionType

    nu = len(prep['minv_stacks'])
    rhos, epss = prep['rhos'], prep['epss']
    cnt = float(B * Nt)

    nc = bacc.Bacc("TRN2", target_bir_lowering=False, debug=False,
                   num_devices=NCORE)

    AB_d = nc.dram_tensor("AB", [128, 8, 2048], bf16, kind="ExternalInput")
    ATR_d = nc.dram_tensor("ATR", [128, 32, 512], bf16, kind="ExternalInput")
    MINV_d = nc.dram_tensor("MINVS", [nu, 128, 8, 512], bf16, kind="ExternalInput")
    WTS_d = nc.dram_tensor("WTS", [128, WTS_W], f32r, kind="ExternalInput")
    WTB_d = nc.dram_tensor("WTB", [128, WTB_W], bf16, kind="ExternalInput")
    CF_d = nc.dram_tensor("CF", [128, 8], f32, kind="ExternalInput")
    FBC_d = nc.dram_tensor("FBC", [128, nu], f32, kind="ExternalInput")
    Y_d = nc.dram_tensor("YBM", [32, 512], f32r, kind="ExternalInput")
    YLH_d = nc.dram_tensor("YLH", [128, 8, 32], bf16, kind="ExternalInput")
    YLN_d = nc.dram_tensor("YLN", [128, 8, 32], bf16, kind="ExternalInput")
    XO_d = nc.dram_tensor("XOUT", [32, 2048], f32r, kind="ExternalOutput")

    with tile.TileContext(nc) as tc:
        with (
            tc.tile_pool(name="cst", bufs=1) as cst,
            tc.tile_pool(name="st", bufs=1) as stp,
            tc.tile_pool(name="cnn", bufs=1) as cnnp,
            tc.tile_pool(name="xin", bufs=4) as xinp,
            tc.tile_pool(name="psbig", bufs=2, space="PSUM") as psbig,
            tc.tile_pool(name="psgen", bufs=2, space="PSUM") as psgen,
            tc.tile_pool(name="psarc", bufs=1, space="PSUM") as psarc,
            tc.tile_pool(name="pssm", bufs=3, space="PSUM") as pssm,
            tc.tile_pool(name="ddr", bufs=2, space="DRAM") as ddr,
        ):
            # ---- constants ----
            ab = cst.tile([128, 8, 2048], bf16, tag="ab")
            atr = cst.tile([128, 32, 512], bf16, tag="atr")
            minv = cst.tile([128, 8, 512], bf16, tag="minv")
            wts = cst.tile([128, WTS_W], f32r, tag="wts")
            wtb = cst.tile([128, WTB_W], bf16, tag="wtb")
            cf = cst.tile([128, 8], f32, tag="cf")
            fbc = cst.tile([128, nu], f32, tag="fbc")
            ylh = cst.tile([128, 8, 32], bf16, tag="ylh")
            yln = cst.tile([128, 8, 32], bf16, tag="yln")
            nc.sync.dma_start(ab[:], AB_d[:])
            nc.sync.dma_start(atr[:], ATR_d[:])
            nc.sync.dma_start(wts[:], WTS_d[:])
            nc.sync.dma_start(wtb[:], WTB_d[:])
            nc.sync.dma_start(cf[:], CF_d[:])
            nc.sync.dma_start(fbc[:], FBC_d[:])
            nc.sync.dma_start(ylh[:], YLH_d[:])
            nc.sync.dma_start(yln[:], YLN_d[:])
            if nu == 1:
                nc.sync.dma_start(minv[:], MINV_d[0])

            OSEL = wts[:, OSEL_C:OSEL_C + 32]
            SELB = wts[0:32, SELB_C:SELB_C + 128]
            IDJN = wts[0:32, IDJN_C:IDJN_C + 64]
            IDJH = wts[0:32, IDJH_C:IDJH_C + 64]
            P32 = wts[0:32, P32_C:P32_C + 32]
            ID32 = wts[0:32, ID32_C:ID32_C + 32]
            ONE128 = wts[0:1, ONES_C:ONES_C + 128]
            ONE11 = wts[0:1, ONES_C:ONES_C + 1]
            W1 = wtb[0:24, W1_C:W1_C + 128]
            W2 = [wtb[:, W2_C + 128 * d: W2_C + 128 * (d + 1)] for d in range(3)]
            WF = [[wtb[:, WF_C + (bp * 3 + d) * 32: WF_C + (bp * 3 + d) * 32 + 32]
                   for d in range(3)] for bp in range(4)]
            g32 = [cf[0:32, 0:1], cf[0:32, 2:3]]
            b32 = [cf[0:32, 1:2], cf[0:32, 3:4]]
            zb128 = cf[:, 5:6]
            zb32 = cf[0:32, 5:6]
            zb1 = cf[0:1, 5:6]
            epsb = cf[0:32, 6:7]

            # ---- state ----
            X = stp.tile([32, 2048], f32r, tag="X")
            RC = stp.tile([32, 2048], f32r, tag="RC")
            XB = stp.tile([32, 2050], bf16, tag="XB")
            S = stp.tile([32, 8, 512], f32r, tag="S")
            y_t, qb4_t, wb_t, marc_t = S[:, 0, :], S[:, 1, :], S[:, 2, :], S[:, 3, :]
            q4_t, t1_t, dif_t, sq_t = S[:, 4, :], S[:, 5, :], S[:, 6, :], S[:, 7, :]
            S2 = stp.tile([32, 2, 512], f32r, tag="S2")
            my_t, md_t = S2[:, 0, :], S2[:, 1, :]
            arcS = stp.tile([32, 512], f32r, tag="arcS")
            s32f = stp.tile([32, 1], f32, tag="s32f")
            facs = stp.tile([32, 4], f32, tag="facs")
            smal = stp.tile([32, 8], f32r, tag="smal")
            gbt = stp.tile([32, 2], f32r, tag="gbt")
            gbb = stp.tile([128, 2], f32, tag="gbb")
            difT = stp.tile([128, 8, 32], bf16, tag="difT")
            wT = stp.tile([128, 8, 32], bf16, tag="wT")
            frow = stp.tile([1, 8, 32], f32, tag="frow")
            fcolS = stp.tile([32, 4], f32, tag="fcolS")
            rcT = stp.tile([128, 32, 32], bf16, tag="rcT")
            stat = cnnp.tile([128, 32], f32, tag="stat")
            stat2 = cnnp.tile([128, 2], f32, tag="stat2")
            stat2r = cnnp.tile([128, 2], f32r, tag="stat2r")
            p32s = cnnp.tile([32, 2], f32r, tag="p32s")
            sdma = cnnp.tile([2, 32], f32, tag="sdma")
            st2 = cnnp.tile([2, 32], f32r, tag="st2")
            agg = cnnp.tile([2, 8, 32], f32, tag="agg")

            A1 = cnnp.tile([128, 4, 2050], bf16, tag="A1")
            A2 = cnnp.tile([128, 4, 2050], bf16, tag="A2")
            SQD = cnnp.tile([128, 2, 512], bf16, tag="SQD")

            nc.sync.dma_start(y_t[:], Y_d[:])
            nc.vector.memset(XB[:, 0:1].bitcast(dt.uint16), 0)
            nc.vector.memset(XB[:, 2049:2050].bitcast(dt.uint16), 0)
            for a in (A1, A2):
                nc.vector.memset(a[:, :, 0:1].bitcast(dt.uint16), 0)
                nc.vector.memset(a[:, :, 2049:2050].bitcast(dt.uint16), 0)

            # ---- x0 = A^H y ----
            for ntb in range(4):
                p = psgen.tile([32, 512], f32, tag="mm")
                for k in range(8):
                    nc.tensor.matmul(p[:], ylh[:, k, :],
                                     ab[:, k, 512 * ntb:512 * (ntb + 1)],
                                     start=(k == 0), stop=(k == 7))
                nc.vector.tensor_copy(X[:, 512 * ntb:512 * (ntb + 1)], p[:])

            def combo_pair(dst, pair, src_view, idj):
                # plain matmul: out = src^T @ [I|J] (is_transpose ignores the
                # identity's values, so signed J requires the normal datapath);
                # top/bottom slots adjacent -> single cast copy
                pT = pssm.tile([128, 64], f32, tag="sm")
                nc.tensor.matmul(pT[:], src_view, idj, start=True, stop=True)
                nc.vector.tensor_copy(
                    dst[:, 2 * pair:2 * pair + 2, :],
                    pT[:].rearrange("p (a b) -> p a b", a=2))

            def bn_collective(layer, acts):
                with nc.allow_low_precision(reason="bn stat reduce"):
                    nc.vector.tensor_reduce(stat2[:, 0:1], stat[:, 0:16], AX.X, OP.add)
                    nc.vector.tensor_reduce(stat2[:, 1:2], stat[:, 16:32], AX.X, OP.add)
                nc.vector.tensor_copy(stat2r[:], stat2[:])
                p = pssm.tile([32, 2], f32, tag="sm")
                nc.tensor.matmul(p[:], OSEL, stat2r[:], start=True, stop=True)
                with nc.allow_low_precision(reason="pack"):
                    nc.vector.tensor_copy(p32s[:], p[:])
                pt = pssm.tile([2, 32], f32, tag="sm")
                nc.tensor.matmul(pt[:], p32s[:], ID32, start=True, stop=True)
                nc.vector.tensor_copy(sdma[:], pt[:])
                ci_ = ddr.tile([2, 32], f32, tag="cc")
                if USE_AG:
                    co_ = ddr.tile([8, 2, 32], f32, tag="cc2")
                    nc.sync.dma_start(ci_[:], sdma[:])
                    nc.gpsimd.collective_compute(
                        "AllGather", OP.bypass,
                        replica_groups=[list(range(NCORE))],
                        ins=[ci_.opt()], outs=[co_.opt()])
                    nc.sync.dma_start(agg[:], co_[:].rearrange("c s v -> s c v"))
                    with nc.allow_low_precision(reason="bn stat reduce"):
                        nc.vector.tensor_reduce(
                            st2[:], agg[:].rearrange("s c v -> s v c"),
                            AX.X, OP.add)
                else:
                    co1 = ddr.tile([2, 32], f32, tag="cc3")
                    nc.sync.dma_start(ci_[:], sdma[:])
                    nc.gpsimd.collective_compute(
                        "AllReduce", OP.add,
                        replica_groups=[list(range(NCORE))],
                        ins=[ci_.opt()], outs=[co1.opt()])
                    nc.sync.dma_start(st2[:].bitcast(f32), co1[:])
                pb = pssm.tile([32, 32], f32, tag="sm")
                nc.tensor.matmul(pb[:, 0:2], st2[:], ID32[0:2, 0:2],
                                 start=True, stop=True)
                with nc.allow_low_precision(reason="bn scalar math in f32r"):
                    nc.vector.tensor_scalar_mul(smal[:, 0:1], pb[:, 0:1], 1.0 / cnt)
                    nc.vector.tensor_scalar_mul(smal[:, 1:2], pb[:, 1:2], 1.0 / cnt)
                    nc.vector.tensor_mul(smal[:, 2:3], smal[:, 0:1], smal[:, 0:1])
                    nc.vector.tensor_sub(smal[:, 3:4], smal[:, 1:2], smal[:, 2:3])
                    nc.scalar.activation(smal[:, 3:4], smal[:, 3:4], AF.Sqrt,
                                         bias=epsb)
                    nc.vector.reciprocal(smal[:, 3:4], smal[:, 3:4])
                    nc.vector.tensor_mul(gbt[:, 0:1], g32[layer], smal[:, 3:4])
                    nc.vector.tensor_mul(smal[:, 2:3], smal[:, 0:1], gbt[:, 0:1])
                    nc.vector.tensor_sub(gbt[:, 1:2], b32[layer], smal[:, 2:3])
                p2 = pssm.tile([128, 2], f32, tag="sm")
                nc.tensor.matmul(p2[:], SELB, gbt[:], start=True, stop=True)
                nc.vector.tensor_copy(gbb[:], p2[:])
                for bp in range(2):
                    nc.scalar.activation(acts[:, bp, 1:2049], acts[:, bp, 1:2049],
                                         AF.Relu, bias=gbb[:, 1:2],
                                         scale=gbb[:, 0:1])
                for bp in range(2, 4):
                    av = acts[:, bp, 1:2049]
                    nc.vector.tensor_scalar(av, av, gbb[:, 0:1], gbb[:, 1:2],
                                            OP.mult, OP.add)
                    nc.vector.tensor_scalar(av, av, 0.0, None, OP.max)

            # ================= iterations =================
            for it in range(ITERS):
                rho = float(rhos[it])
                eps = float(epss[it])
                c1 = 1.0 / (rho + 1e-8)
                uidx = prep['iter_minv_idx'][it]
                if nu > 1:
                    nc.sync.dma_start(minv[:], MINV_d[uidx])

                # ---- CNN: xb cast + gather ----
                nc.vector.tensor_copy(XB[:, 1:2049], X[:])
                xins = []
                for bp in range(4):
                    xin = xinp.tile([24, 2048], bf16, tag="xin")
                    for dl in range(3):
                        for ci in range(2):
                            eng = nc.sync if (dl % 2 == 0) else nc.scalar
                            eng.dma_start(
                                xin[dl * 8 + ci * 4: dl * 8 + ci * 4 + 4, :],
                                XB[ci * 16 + bp * 4: ci * 16 + bp * 4 + 4,
                                   dl:dl + 2048])
                    xins.append(xin)
                # conv1
                for lt in range(4):
                    for bp in range(4):
                        pb_ = psbig.tile([128, 512], f32, tag="big")
                        nc.tensor.matmul(
                            pb_[:], W1,
                            xins[bp][0:24, 512 * lt:512 * (lt + 1)],
                            start=True, stop=True)
                        g = lt * 4 + bp
                        a1v = A1[:, bp, 1 + 512 * lt:1 + 512 * (lt + 1)]
                        nc.scalar.activation(a1v, pb_[:], AF.Identity,
                                             bias=zb128,
                                             accum_out=stat[:, g:g + 1])
                        nc.vector.scalar_tensor_tensor(
                            SQD[:, 0, :], a1v, 1.0, a1v, OP.mult, OP.mult,
                            accum_out=stat[:, 16 + g:17 + g])

                # MY = Minv@y and MD0 = Minv@diffT_last: run on PE during
                # the bn1 collective stall
                pmy = psgen.tile([32, 512], f32, tag="mm")
                for k in range(8):
                    nc.tensor.matmul(pmy[:], yln[:, k, :], minv[:, k, :],
                                     start=(k == 0), stop=(k == 7))
                nc.vector.tensor_copy(my_t[:], pmy[:])
                if it > 0:
                    pmd = psgen.tile([32, 512], f32, tag="mm")
                    for k in range(8):
                        nc.tensor.matmul(pmd[:], difT[:, k, :], minv[:, k, :],
                                         start=(k == 0), stop=(k == 7))
                    nc.vector.tensor_copy(md_t[:], pmd[:])

                bn_collective(0, A1)

                # conv2
                for lt in range(4):
                    for bp in range(4):
                        pb_ = psbig.tile([128, 512], f32, tag="big")
                        for dl in range(3):
                            nc.tensor.matmul(
                                pb_[:], W2[dl],
                                A1[:, bp, dl + 512 * lt: dl + 512 * lt + 512],
                                start=(dl == 0), stop=(dl == 2))
                        g = lt * 4 + bp
                        a2v = A2[:, bp, 1 + 512 * lt:1 + 512 * (lt + 1)]
                        nc.scalar.activation(a2v, pb_[:], AF.Identity,
                                             bias=zb128,
                                             accum_out=stat[:, g:g + 1])
                        nc.vector.scalar_tensor_tensor(
                            SQD[:, 0, :], a2v, 1.0, a2v, OP.mult, OP.mult,
                            accum_out=stat[:, 16 + g:17 + g])
                bn_collective(1, A2)

                # convf + rc1 + comboN(rc1) + arc matmuls
                parc = psarc.tile([32, 512], f32, tag="arc")
                first_mm = True
                for lt in range(4):
                    pcf = psgen.tile([32, 512], f32, tag="mm")
                    nmm = 0
                    for bp in range(4):
                        for dl in range(3):
                            nc.tensor.matmul(
                                pcf[:], WF[bp][dl],
                                A2[:, bp, dl + 512 * lt: dl + 512 * lt + 512],
                                start=(nmm == 0), stop=(nmm == 11))
                            nmm += 1
                    blk = slice(512 * lt, 512 * (lt + 1))
                    # rc1 = (pcf + x)*c1 + fb*c1
                    nc.vector.scalar_tensor_tensor(
                        sq_t[:], pcf[:], 1.0, X[:, blk], OP.mult, OP.add)
                    nc.scalar.activation(RC[:, blk], sq_t[:], AF.Identity,
                                         bias=fbc[0:32, uidx:uidx + 1], scale=c1)
                    for c in range(4):
                        combo_pair(rcT, lt * 4 + c,
                                   RC[:, 512 * lt + 128 * c:
                                      512 * lt + 128 * (c + 1)], IDJN)
                    for c in range(4):
                        tc_ = lt * 4 + c
                        nc.tensor.matmul(parc[:], rcT[:, 2 * tc_, :],
                                         atr[:, 2 * tc_, :],
                                         start=first_mm, stop=False)
                        first_mm = False
                        last = (lt == 3 and c == 3)
                        nc.tensor.matmul(parc[:], rcT[:, 2 * tc_ + 1, :],
                                         atr[:, 2 * tc_ + 1, :],
                                         start=False, stop=last)
                nc.vector.tensor_copy(arcS[:], parc[:])
                # comboN(arc) -> wT scratch; marc = Minv @ arc
                for c in range(4):
                    combo_pair(wT, c, arcS[:, 128 * c:128 * (c + 1)], IDJN)
                pma = psgen.tile([32, 512], f32, tag="mm")
                for k in range(8):
                    nc.tensor.matmul(pma[:], wT[:, k, :], minv[:, k, :],
                                     start=(k == 0), stop=(k == 7))
                nc.vector.tensor_copy(marc_t[:], pma[:])

                # ---------- ADMM steps ----------
                r2c1 = rho * rho * c1
                r3c1 = rho * rho * rho * c1
                # qb4 = rho*marc + (rho^2*c1-1)*y - rho^3*c1*MY ; wbase
                nc.vector.tensor_scalar(sq_t[:], y_t[:], r2c1 - 1.0, None,
                                        OP.mult)
                nc.vector.scalar_tensor_tensor(
                    sq_t[:], marc_t[:], rho, sq_t[:], OP.mult, OP.add)
                nc.vector.scalar_tensor_tensor(
                    qb4_t[:], my_t[:], -r3c1, sq_t[:], OP.mult, OP.add)
                nc.vector.scalar_tensor_tensor(
                    wb_t[:], my_t[:], r2c1, marc_t[:], OP.mult, OP.subtract)
                if it > 0:
                    # q4(s0) = qb4 + gamma0*diff_last
                    nc.vector.scalar_tensor_tensor(
                        q4_t[:], dif_t[:], fcolS[:, 0:1], qb4_t[:],
                        OP.mult, OP.add)
                for s in range(ADMM):
                    final = (s == ADMM - 1)
                    last_all = (it == ITERS - 1 and final)
                    # diff = q4 + kf*MDprev  (s0 it0: diff = qb4)
                    if it == 0 and s == 0:
                        nc.vector.tensor_copy(dif_t[:], qb4_t[:])
                    elif s == 0:
                        nc.vector.scalar_tensor_tensor(
                            dif_t[:], md_t[:], fcolS[:, 1:2], q4_t[:],
                            OP.mult, OP.add)
                    else:
                        nc.vector.scalar_tensor_tensor(
                            dif_t[:], pmd_s[:], kf_ap, q4_t[:],
                            OP.mult, OP.add)
                    if final:
                        # w = wbase + (r2c1*fac3_s1)*MD(s1)
                        nc.vector.scalar_tensor_tensor(
                            t1_t[:], pmd_s[:], fcolS[:, 2:3], wb_t[:],
                            OP.mult, OP.add)
                    # diffT combos + MD MMs (PE, overlap norm chain)
                    if not last_all:
                        for c in range(4):
                            combo_pair(difT, c,
                                       dif_t[:, 128 * c:128 * (c + 1)], IDJN)
                    if not final:
                        pmd_s = psgen.tile([32, 512], f32, tag="mm")
                        for k in range(8):
                            nc.tensor.matmul(pmd_s[:], difT[:, k, :],
                                             minv[:, k, :],
                                             start=(k == 0), stop=(k == 7))
                    # norm chain in [1,32] row space
                    nc.vector.scalar_tensor_tensor(
                        sq_t[:], dif_t[:], 1.0, dif_t[:], OP.mult, OP.mult,
                        accum_out=s32f[:])
                    pn = pssm.tile([1, 32], f32, tag="sm")
                    nc.tensor.matmul(pn[:], s32f[:], P32.bitcast(f32),
                                     start=True, stop=True)
                    nc.scalar.activation(frow[0:1, 0, :], pn[:], AF.Sqrt,
                                         bias=zb1)
                    nc.vector.reciprocal(frow[0:1, 0, :], frow[0:1, 0, :])
                    nc.vector.tensor_scalar(frow[0:1, 1, :], frow[0:1, 0, :],
                                            eps, 1.0, OP.mult, OP.min)
                    fr = frow[0:1, 1, :]
                    def fcol(slot, a_, b_, dst):
                        # dst [32,1] <- fac*a_ + b_ broadcast to B-space col
                        nc.vector.tensor_scalar(frow[0:1, slot, :], fr,
                                                a_, b_, OP.mult, OP.add)
                        pf_ = pssm.tile([32, 1], f32, tag="sm")
                        nc.tensor.matmul(pf_[:], frow[0:1, slot, :],
                                         ONE11.bitcast(f32),
                                         start=True, stop=True)
                        nc.vector.tensor_copy(dst, pf_[:])
                        return pf_
                    if final:
                        if not last_all:
                            # gamma0 = fac*(r2c1-1)+(1-r2c1); kf0 = r3c1*(1-fac)
                            fcol(2, r2c1 - 1.0, 1.0 - r2c1, fcolS[:, 0:1])
                            fcol(3, -r3c1, r3c1, fcolS[:, 1:2])
                    else:
                        # gamma = fac*(2*r2c1-1)+(1-r2c1); kf = -2*r3c1*fac+r3c1
                        fcol(2, 2.0 * r2c1 - 1.0, 1.0 - r2c1, fcolS[:, 3:4])
                        fcol(3, -2.0 * r3c1, r3c1, fcolS[:, 1:2])
                        kf_ap = fcolS[:, 1:2]
                        if s == ADMM - 2:
                            # wf = r2c1*(2fac-1)
                            fcol(4, 2.0 * r2c1, -r2c1, fcolS[:, 2:3])
                        # q4' = qb4 + gamma*diff
                        nc.vector.scalar_tensor_tensor(
                            q4_t[:], dif_t[:], fcolS[:, 3:4], qb4_t[:],
                            OP.mult, OP.add)
                    if final:
                        for c in range(4):
                            combo_pair(wT, c,
                                       t1_t[:, 128 * c:128 * (c + 1)], IDJH)
                        for ntb in range(4):
                            px = psgen.tile([32, 512], f32, tag="mm")
                            for k in range(8):
                                nc.tensor.matmul(
                                    px[:], wT[:, k, :],
                                    ab[:, k, 512 * ntb:512 * (ntb + 1)],
                                    start=(k == 0), stop=(k == 7))
                            blk = slice(512 * ntb, 512 * (ntb + 1))
                            nc.vector.tensor_add(X[:, blk], px[:], RC[:, blk])

            nc.sync.dma_start(XO_d[:], X[:])

    nc.compile()
    return nc


def _enable_trace_shim():
    import sys, types
    try:
        import trn_agent_boot.trn_boot as _tb
        import concourse.bass_utils as _bu
        _bu.upload_artifacts = lambda tmpdir: "local://" + str(tmpdir)
        hookmod = types.ModuleType('antenv.axon_hooks')
        hook = _tb._ntff_profile_via_ctypes('/opt/axon/libaxon_pjrt.so')
        hookmod.get_axon_ntff_profile_hook = lambda: hook
        import antenv as _antenv
        sys.modules['antenv.axon_hooks'] = hookmod
        _antenv.axon_hooks = hookmod
        return True
    except Exception:
        return False


def kernel(**inputs) -> np.ndarray:
    import os
    from concourse.bass_utils import run_bass_kernel_spmd
    trace = bool(os.environ.get("KERNEL_TRACE"))
    if trace:
        trace = _enable_trace_shim()

    prep = _host_prep(inputs)
    nc = _build_program(prep)

    minvs = np.stack(prep['minv_stacks'], 0)
    in_maps = []
    for c in range(NCORE):
        in_maps.append({
            "AB": prep['AB'], "ATR": prep['ATR'], "MINVS": minvs,
            "WTS": prep['WTS'], "WTB": prep['WTB'], "CF": prep['CF'],
            "FBC": prep['FBC'],
            "YBM": np.ascontiguousarray(prep['ybm_cores'][c][:, :512]),
            "YLH": prep['ylh_cores'][c], "YLN": prep['yln_cores'][c],
        })
    res = run_bass_kernel_spmd(nc, in_maps, list(range(NCORE)), trace=trace)
    out = np.zeros((B, 2, Nt), np.float32)
    for c in range(NCORE):
        xc = res.results[c]["XOUT"]
        out[c * BS:(c + 1) * BS, 0] = xc[:16]
        out[c * BS:(c + 1) * BS, 1] = xc[16:]
    kernel._last_results = res
    return out
